# revision 99
# baseline (speedup 1.0000x reference)
"""Causal varlen self-attention (packed, equal-length) on 8 trn2 NeuronCores.

Sharding: tensor-parallel over heads — 16 heads / 8 cores = 2 heads per core.
Each core computes qkv + RoPE + RMSNorm + causal attention + sigmoid gating for
its 2 heads over all 4096 tokens, plus its partial output projection
(attn_chunk @ Wo_chunk.T).  The host sums the 8 partial outputs.

Matmul precision: the big projections (QKV, Wo) run as error-compensated fp8
DoubleRow matmuls: W = Whi + Wlo and x = xhi + xlo (each plane e4m3), and the
product is Whi·xhi + Whi·xlo + Wlo·xhi (the lo·lo term is ~0.03% and dropped).
That is 24 DR matmuls per 2048-deep contraction instead of 16 bf16 matmuls —
0.75x the PE time at slightly BETTER than bf16 accuracy.  Scales: x-planes
carry 2x, W-planes carry 64x, so qkv psum is 128x true.  q/k are RMS-
normalized downstream (scale cancels); the v/gate 128x folds into the gate
activation constants; the Wo result's 512x is divided out in the host-side
partial-sum reduction.

Per-core pipeline (feature-major q/k: head_dim on partitions):
  - qkv: q,k produced feature-major [d, t]; v (+ the 2 gate logits appended as
    2 extra columns of the v weight block) produced token-major [t, d].
  - RoPE via elementwise muls on a half-swapped copy; RMSNorm partition-
    reductions via ones-matmuls on the PE.
  - scores computed TRANSPOSED: scoresT[s, t] = k_fin-slices.T @ q_fin so the
    k-side softmax scale folds into the exp's per-partition scale, and the
    transposed probs are exactly what the PV matmul (lhsT = token-major V)
    wants.  Softmax denominator = ones-matmul over the exp tiles.
  - causal mask: diagonal-chunk matmuls are sliced to the unmasked t-range and
    one [128,128] triangle of -1e30 is added before exp.
  - gate and 1/denominator are per-token (free-dim) scales, applied via a
    partition-broadcast SBUF->SBUF DMA then one elementwise multiply.
"""

import sys

sys.path.insert(0, "/opt/trn_rl_repo")

import numpy as np
import ml_dtypes

import concourse.bass as bass
import concourse.tile as tile
from concourse import bacc, mybir
from concourse.bass_utils import run_bass_kernel_spmd

N_TOK, HID, NH, HD = 4096, 2048, 16, 128
SEQ, NSEQ = 1024, 4
NCORES = 8
EPS = 1e-6
F32, BF16, F32R = mybir.dt.float32, mybir.dt.bfloat16, mybir.dt.float32r
FP8 = mybir.dt.float8e4
BF = ml_dtypes.bfloat16
E4 = ml_dtypes.float8_e4m3
AF = mybir.ActivationFunctionType
DR = mybir.MatmulPerfMode.DoubleRow

SX = 2.0     # x-plane scale
SW = 64.0    # W-plane scale (qkv, wo)
SQKV = SX * SW          # qkv psum scale (128)
SATT = 8.0              # att sbuf scale
SOUT = SATT * SW        # wo psum scale (512), divided out on the host

_PATCHED = False
_ACT_PATCHED = False


def _patch_act_tables():
    """Force every ACT-table choice onto the one table that holds BOTH exp
    and ln (plus copy/identity/square): the load-insertion pass picks the
    first table containing each required func, and with distinct tables for
    Exp vs Ln the scheduler's interleaving causes a ~1.3us LoadActFuncSet
    per alternation.  Emptying every other entry (indices preserved, so the
    emitted act_func_set_id still matches act_info.json) makes the combined
    table the unique choice: exactly one load for the whole kernel."""
    global _ACT_PATCHED
    if _ACT_PATCHED:
        return
    _ACT_PATCHED = True
    import concourse.bacc as _bacc
    import concourse.bass_interp as _bi
    from concourse import mybir as _mb
    from concourse.hw_specs import get_activation_tables as _orig

    def patched(arch):
        tabs = _orig(arch)
        combo = {
            name
            for name, s in tabs.items()
            if _mb.ActivationFunctionType.Exp in s
            and _mb.ActivationFunctionType.Ln in s
        }
        assert combo, "no table with both Exp and Ln"
        return {
            name: (s if name in combo else set()) for name, s in tabs.items()
        }

    _bacc.get_activation_tables = patched
    _bi.get_activation_tables = patched


def _patch_tile_drain():
    """walrus in this env allows only ONE sync-wait on a TPB_CTRL instruction;
    spread the TileContext-exit drain's waits across nop instructions."""
    global _PATCHED
    if _PATCHED:
        return
    _PATCHED = True
    from concourse.tile import TileContext
    from concourse.vector_clock import ScopedClock

    def patched(self, tick_clock, wait_clock):
        nc = self.nc
        probe = nc.sync.nop(nofuse=True, hint="drain_waits_probe")
        wait_clock.add_sem_waits(probe.ins, ScopedClock({None: tick_clock.global_clock}))
        raw = list(probe.ins.sync_info.on_wait or [])
        best = {}
        for w in raw:  # keep one wait per semaphore (the largest threshold)
            k = (w.id, w.wait_mode)
            if k not in best or (w.wait_value or 0) > (best[k].wait_value or 0):
                best[k] = w
        waits = list(best.values())
        probe.ins.sync_info.on_wait = waits[:1]
        for w in waits[1:]:
            nop = nc.sync.nop(nofuse=True, hint="drain_waits")
            nop.ins.sync_info = mybir.SyncInfo(on_wait=[w], on_update=[])
        nc.sync.drain()
        nc.all_engine_barrier()
        assert self.sems is not None
        popped = nc._tile_sem_poison_stack.pop()
        assert popped is self._sem_poison
        nc.clear_and_free_semaphores(list(self.sems.allocated().values()))
        nc.all_engine_barrier()

    TileContext._drain_and_barrier = patched


def _r(ap):
    return ap.bitcast(F32R)


def build_nc():
    """One SPMD Bass program; all per-core data arrives via ExternalInputs."""
    _patch_act_tables()
    nc = bacc.Bacc("TRN2", target_bir_lowering=False, debug=False, num_devices=NCORES)

    # dim1 of xt/wqk/wvg/wot: 0 = hi fp8 plane, 1 = lo (residual) plane
    # xt dim2 = the 8 512-token tiles so a tile load is one contiguous
    # per-partition slab (DMA cost is row-count + bytes; 512B rows are 3x
    # slower than byte rate)
    xt = nc.dram_tensor("xt", [128, 2, 8, 16, 512], FP8, kind="ExternalInput")
    wqk = nc.dram_tensor("wqk", [128, 2, 4, 16, 128], FP8, kind="ExternalInput")
    wvg = nc.dram_tensor("wvg", [128, 2, 16, 258], FP8, kind="ExternalInput")
    wot = nc.dram_tensor("wot", [128, 2, 2, HID], FP8, kind="ExternalInput")
    cs = nc.dram_tensor("cs", [128, 2, SEQ], BF16, kind="ExternalInput")
    csk = nc.dram_tensor("csk", [128, 2, SEQ], BF16, kind="ExternalInput")
    tri = nc.dram_tensor("tri", [128, 128], BF16, kind="ExternalInput")
    idn = nc.dram_tensor("idn", [128, 128], BF16, kind="ExternalInput")
    gbc = nc.dram_tensor("gbc", [128, 2], F32, kind="ExternalInput")
    out = nc.dram_tensor("out", [N_TOK, HID], BF16, kind="ExternalOutput")
    gate_scr = nc.dram_tensor("gate_scr", [NSEQ, 2, 8, 128], BF16)

    with tile.TileContext(nc) as tc:
        with (
            tc.tile_pool(name="consts", bufs=1) as consts,
            tc.tile_pool(name="xtp", bufs=3) as xtp,
            tc.tile_pool(name="qkp", bufs=2) as qkp,
            tc.tile_pool(name="vp", bufs=2) as vp,
            tc.tile_pool(name="ropep", bufs=5) as ropep,
            tc.tile_pool(name="scrp", bufs=2) as scrp,
            tc.tile_pool(name="expp", bufs=3) as expp,
            tc.tile_pool(name="attnp", bufs=2) as attnp,
            tc.tile_pool(name="outp", bufs=3) as outp,
            tc.tile_pool(name="bcp", bufs=2) as bcp,
            tc.tile_pool(name="rowp", bufs=2) as rowp,
            tc.tile_pool(name="gsp", bufs=2) as gsp,
            tc.tile_pool(name="projps", bufs=2, space="PSUM") as projps,
            tc.tile_pool(name="bigps", bufs=3, space="PSUM") as bigps,
            tc.tile_pool(name="pvps", bufs=2, space="PSUM") as pvps,
            tc.tile_pool(name="vecps", bufs=1, space="PSUM") as vecps,
        ):
            # ---- resident constants
            wqk_t = consts.tile([128, 2, 4, 16, 128], FP8)
            wvg_t = consts.tile([128, 2, 16, 258], FP8)
            wot_t = consts.tile([128, 2, 2, HID], FP8)
            cs_t = consts.tile([128, 2, SEQ], BF16)
            csk_t = consts.tile([128, 2, SEQ], BF16)
            tri_t = consts.tile([128, 128], BF16)
            idn_t = consts.tile([128, 128], BF16)
            gbn_t = consts.tile([128, 2], F32)

            def late_consts():
                # on the same (sync) queue as the startup stream, emitted
                # after it: a second queue's HWDGE generations would
                # interleave 1:1 and dilute the startup bytes
                nc.sync.dma_start(out=cs_t[:], in_=cs[:])
                nc.sync.dma_start(out=csk_t[:], in_=csk[:])
                nc.sync.dma_start(out=tri_t[:], in_=tri[:])
                nc.sync.dma_start(out=idn_t[:], in_=idn[:])
                nc.sync.dma_start(out=gbn_t[:], in_=gbc[:])
                nc.sync.dma_start(out=wot_t[:], in_=wot[:])
            ones_t = consts.tile([128, 1], F32)
            nc.vector.memset(ones_t[:], 1.0)
            ones_bf = consts.tile([128, 1], BF16)
            nc.vector.memset(ones_bf[:], 1.0)
            # ones/HD in bf16 (2^-7, exact): the q-stats matmul yields mean_d
            ones_q = consts.tile([128, 1], BF16)
            nc.vector.memset(ones_q[:], 1.0 / HD)
            eps_t = consts.tile([128, 1], F32)
            nc.vector.memset(eps_t[:], EPS)
            epsh_t = consts.tile([128, 1], F32)
            # k-side stats run on SQKV-scaled k: eps scales by SQKV^2
            nc.vector.memset(epsh_t[:], float(HD * EPS * SQKV * SQKV))
            c16_t = consts.tile([128, 1], F32)
            nc.vector.memset(c16_t[:], float(SQKV / SATT))

            def qkv_mm(ps, m, wp, xp, c, start, stop):
                nc.tensor.matmul(
                    ps[:],
                    lhsT=wqk_t[:, wp, m, 2 * c : 2 * c + 2, :],
                    rhs=xp[:, 2 * c : 2 * c + 2, :],
                    start=start, stop=stop, perf_mode=DR,
                )

            def v_mm(ps, ti, wp, xp, c, start, stop):
                nc.tensor.matmul(
                    ps[:, 0:258],
                    lhsT=xp[:, 2 * c : 2 * c + 2, ti * 128 : (ti + 1) * 128],
                    rhs=wvg_t[:, wp, 2 * c : 2 * c + 2, :],
                    start=start, stop=stop, perf_mode=DR,
                )

            def qk_copy(qk, m, half, ps):
                dst = qk[:, m, half * 512 : (half + 1) * 512]
                with tc.high_priority():
                    nc.vector.tensor_copy(out=dst, in_=ps[:])

            def v_copy(vt, gst, half, ti, ps):
                with tc.high_priority():
                    nc.vector.tensor_copy(
                        out=vt[:, half * 4 + ti, :], in_=ps[:, 0:256])
                # stash the 2 gate-logit columns; exp'd per-seq in gate_fin
                with tc.high_priority():
                    nc.vector.tensor_copy(
                        out=gst[:, :, half * 4 + ti], in_=ps[:, 256:258]
                    )

            def qkv_ntile(nt, qk, vt, gst):
                """project 512 tokens: q,k feature-major; v+gate token-major.
                Each output = Whi.xhi + Whi.xlo + Wlo.xhi via fp8 DoubleRow."""
                half = nt % 2
                xtile = load_xtile(nt)
                xh, xl = xtile[:, 0], xtile[:, 1]
                for m in range(4):  # q_h0, q_h1, k_h0, k_h1
                    ps = projps.tile([128, 512], F32, tag="proj")
                    passes = ((0, xh), (0, xl), (1, xh))
                    for pi, (wp, xp) in enumerate(passes):
                        for c in range(8):
                            qkv_mm(ps, m, wp, xp, c,
                                   pi == 0 and c == 0, pi == 2 and c == 7)
                    qk_copy(qk, m, half, ps)
                for ti in range(4):  # v + gate logits, token-major, 128 tok each
                    ps = projps.tile([128, 512], F32, tag="proj")
                    passes = ((0, xh), (1, xh), (0, xl))
                    for pi, (wp, xp) in enumerate(passes):
                        for c in range(8):
                            v_mm(ps, ti, wp, xp, c,
                                 pi == 0 and c == 0, pi == 2 and c == 7)
                    v_copy(vt, gst, half, ti, ps)

            def qkv_ntile_startup(nt, qk, vt, gst):
                """pass-outer variant for the JIT startup tiles: all m/ti psum
                groups stay open across the three plane passes, so the PE only
                ever waits on the plane the current pass consumes (xhi first,
                then xlo, then the lo weights) instead of on the full tile."""
                half = nt % 2
                xtile = load_xtile(nt)
                xh, xl = xtile[:, 0], xtile[:, 1]
                mps = {}
                for m in range(4):
                    mps[m] = (projps if m < 2 else bigps).tile(
                        [128, 512], F32, tag="proj" if m < 2 else "big",
                        name=f"sqkv{nt}_{m}")
                    for c in range(8):
                        qkv_mm(mps[m], m, 0, xh, c, c == 0, False)
                vps = {}
                for ti in range(4):
                    vps[ti] = (pvps, bigps, vecps, pvps)[ti].tile(
                        [128, 512], F32,
                        tag=("pv", "big", "vec", "pv")[ti],
                        name=f"sv{nt}_{ti}")
                    for c in range(8):
                        v_mm(vps[ti], ti, 0, xh, c, c == 0, False)
                for m in range(4):
                    for c in range(8):
                        qkv_mm(mps[m], m, 0, xl, c, False, False)
                for ti in range(4):
                    for c in range(8):
                        v_mm(vps[ti], ti, 1, xh, c, False, False)
                for m in range(4):
                    for c in range(8):
                        qkv_mm(mps[m], m, 1, xh, c, False, c == 7)
                    qk_copy(qk, m, half, mps[m])
                for ti in range(4):
                    for c in range(8):
                        v_mm(vps[ti], ti, 0, xl, c, False, c == 7)
                    v_copy(vt, gst, half, ti, vps[ti])

            def gate_fin(s, gst):
                """gate rows hold (SQKV/SATT)*(1+exp(-(z+b))) for the whole
                seq, one DMA to DRAM; the reciprocal later turns this into
                SATT*sigmoid/(SQKV*den) so att lands at SATT*true given the
                SQKV-scaled pv."""
                for h in range(2):
                    nc.scalar.activation(
                        out=gst[:, h, :], in_=gst[:, h, :], func=AF.Exp,
                        bias=gbn_t[:, h : h + 1], scale=-1.0 / SQKV,
                    )
                # gst <- (gst + 1) * SQKV/SATT  == Copy((SQKV/SATT)*gst + 16)
                nc.scalar.activation(
                    out=gst[:, :, :], in_=gst[:, :, :], func=AF.Copy,
                    bias=float(SQKV / SATT), scale=float(SQKV / SATT),
                )
                nc.sync.dma_start(
                    out=gate_scr[s].rearrange("h ti p -> p h ti"), in_=gst[:]
                )

            def rope_norm(s, h, is_q, qk):
                """RoPE + RMSNorm scale for one head-tensor of one sequence.
                q: returns fin already scaled by sigma_q (broadcast multiply).
                k: returns (fin * norm_w^2, sigma_k per-partition column).
                All inputs are SQKV-scaled; sigma normalization cancels it."""
                m = h if is_q else 2 + h
                cst = cs_t if is_q else csk_t
                fin = ropep.tile([128, SEQ], BF16, tag="rope")
                sq = scrp.tile([128, SEQ], BF16, tag="sq")
                # rope is a per-pair rotation: it preserves sum_d q^2, so the
                # RMSNorm stats come from PRE-rope values — a chain parallel to
                # the rotation, not serial after it
                with tc.high_priority():
                    nc.vector.tensor_mul(out=sq[:], in0=qk[:, m, :], in1=qk[:, m, :])
                # half-swap via SBUF->SBUF DMA (the only cheap cross-partition
                # path besides the PE); sin plane is [S; -S] so the rotation is
                # then plain partition-aligned elementwise work
                qsw = scrp.tile([128, SEQ], BF16, tag="qsw")
                nc.sync.dma_start(out=qsw[0:64, :], in_=qk[64:128, m, :])
                nc.sync.dma_start(out=qsw[64:128, :], in_=qk[0:64, m, :])
                nc.vector.tensor_mul(out=qsw[:], in0=qsw[:], in1=cst[:, 1, :])
                for j in range(2):
                    js = slice(j * 512, (j + 1) * 512)
                    nc.gpsimd.tensor_mul(
                        out=fin[:, js], in0=qk[:, m, js], in1=cst[:, 0, js]
                    )
                    nc.vector.tensor_add(
                        out=fin[:, js], in0=fin[:, js], in1=qsw[:, js]
                    )
                if is_q:
                    # sigma_q[t] = rsqrt(mean_d(rope_q^2) + eps) computed as
                    # exp(-0.5*ln(x+eps)); the SQKV^2 in mean_d makes eps
                    # relatively smaller, otherwise identical.  free-dim
                    # scale, per 512-half so the first scores tile unblocks
                    # as early as possible
                    row = rowp.tile([1, SEQ], F32, tag="qrow")
                    for j in range(2):
                        js = slice(j * 512, (j + 1) * 512)
                        bc = bcp.tile([128, 512], F32, tag="bcq")
                        pss = vecps.tile([1, 512], F32, tag="vec")
                        nc.tensor.matmul(
                            pss[:],
                            lhsT=ones_q[:],
                            rhs=sq[:, js],
                            start=True,
                            stop=True,
                        )
                        # rsqrt as exp(-0.5*ln(x+eps)): keeps ACT on the one
                        # exp+ln table (zero LoadActFuncSet thrash)
                        nc.scalar.activation(
                            out=row[:, js], in_=pss[:], func=AF.Ln,
                            bias=eps_t[0:1, :], scale=1.0,
                        )
                        nc.scalar.activation(
                            out=row[:, js], in_=row[:, js], func=AF.Exp,
                            scale=-0.5,
                        )
                        nc.gpsimd.partition_broadcast(bc[:], row[:, js])
                        nc.vector.tensor_mul(
                            out=fin[:, js], in0=fin[:, js], in1=bc[:]
                        )
                    return fin, None
                else:
                    # sigma_k[s] = rsqrt(sum_d + HD*eps*SQKV^2), per-partition
                    # column applied inside the exp
                    col = rowp.tile([128, 8], F32, tag="kcol")
                    psc = projps.tile([128, 8], F32, tag="proj")
                    for sc in range(8):
                        nc.tensor.matmul(
                            psc[:, sc : sc + 1],
                            lhsT=sq[:, sc * 128 : (sc + 1) * 128],
                            rhs=ones_bf[:],
                            start=True,
                            stop=True,
                            skip_group_check=True,
                        )
                    nc.scalar.activation(
                        out=col[:], in_=psc[:], func=AF.Ln,
                        bias=epsh_t[:], scale=1.0,
                    )
                    nc.scalar.activation(
                        out=col[:], in_=col[:], func=AF.Exp, scale=-0.5,
                    )
                    return fin, col

            def attention(s, h, qk, vt, atthi, attlo, qf, kf, kcol,
                          grow_t, fill, tts_only=None):
                grows = [grow_t[0:1, h, 4 * tt : 4 * tt + 4, :] for tt in range(2)]
                for tt in range(2) if tts_only is None else tts_only:
                    nsc = 4 * (tt + 1)
                    expt = expp.tile([128, 8, 512], BF16, tag="expt")
                    pv = pvps.tile([128, 512], F32, tag="pv")
                    den = vecps.tile([1, 512], F32, tag="vec")
                    es = scrp.tile([128, 512], BF16, tag="esum")
                    for sc in range(nsc):
                        r = sc - 4 * tt  # >= 0 on diagonal chunks
                        c0 = 128 * r if r > 0 else 0
                        sps = bigps.tile([128, 512], F32, tag="big")
                        nc.tensor.matmul(
                            sps[:, 0 : 512 - c0],
                            lhsT=kf[:, sc * 128 : (sc + 1) * 128],
                            rhs=qf[:, tt * 512 + c0 : (tt + 1) * 512],
                            start=True,
                            stop=(r < 0),
                            skip_group_check=True,
                        )
                        if r >= 0:  # diagonal chunk: accumulate the -1e30
                            # triangle on the PE itself (I.T @ tri) — keeps the
                            # scores->exp chain off the DVE
                            nc.tensor.matmul(
                                sps[:, 0:128],
                                lhsT=idn_t[:],
                                rhs=tri_t[:],
                                start=False,
                                stop=True,
                                skip_group_check=True,
                            )
                        with tc.high_priority():
                            nc.scalar.activation(
                                out=expt[:, sc, c0:512], in_=sps[:, 0 : 512 - c0],
                                func=AF.Exp, scale=kcol[:, sc : sc + 1],
                            )
                        # running bf16 partial sum of exp chunks on the DVE
                        # (2x mode) so the softmax denominator costs one
                        # [1,512] ones-matmul per tt instead of nsc of them
                        with tc.high_priority():
                            if sc == 0:
                                nc.vector.tensor_copy(out=es[:], in_=expt[:, 0, :])
                            else:
                                nc.vector.tensor_add(
                                    out=es[:, c0:512], in0=es[:, c0:512],
                                    in1=expt[:, sc, c0:512],
                                )
                        nc.tensor.matmul(
                            pv[:, c0:512],
                            lhsT=vt[:, sc, h * 128 : (h + 1) * 128],
                            rhs=expt[:, sc, c0:512],
                            start=(sc == 0),
                            stop=(sc == nsc - 1),
                            skip_group_check=True,
                        )
                        fill()
                    nc.tensor.matmul(
                        den[:], lhsT=ones_bf[:], rhs=es[:], start=True, stop=True,
                        skip_group_check=True,
                    )
                    fill()
                    drec = rowp.tile([1, 512], F32, tag="drec")
                    tts = slice(tt * 512, (tt + 1) * 512)
                    att = attnp.tile([128, 512], BF16, tag="attsc")
                    with tc.high_priority():
                        nc.vector.tensor_mul(out=drec[:], in0=den[:], in1=grows[tt][:])
                        nc.vector.reciprocal(out=drec[:], in_=drec[:])
                        bcg = bcp.tile([128, 512], F32, tag="bcg")
                        nc.gpsimd.partition_broadcast(bcg[:], drec[:])
                        nc.vector.tensor_mul(
                            out=att[:], in0=pv[:], in1=bcg[:]
                        )
                    # fp8 split for the DoubleRow Wo: hi = fp8(att),
                    # lo = fp8(att - hi); the sub runs on the (slack) Pool
                    # engine to keep it off the saturated DVE
                    with tc.high_priority():
                        nc.vector.tensor_copy(out=atthi[:, h, tts], in_=att[:])
                    nc.gpsimd.tensor_sub(
                        out=attlo[:, h, tts], in0=att[:],
                        in1=atthi[:, h, tts],
                    )

            def wo_fillers(s, atthi, attlo, last=False):
                """the previous seq's Wo projection as 32 (t8, ot) closures,
                emitted one at a time inside the next seq's attention to keep
                the PE instruction stream free of dependency stalls.  Each
                closure: 3 fp8 DoubleRow matmuls (hi.Whi + hi.Wlo + lo.Whi),
                contracting both heads at once via the [128, 2, t] att AP."""
                obs = {}
                fillers = []
                for t8 in range(8):
                    for ot in range(4):
                        def emit(t8=t8, ot=ot):
                            if ot == 0:
                                obs[t8] = outp.tile([128, 4, 512], BF16, tag="ob", name=f"ob{s}_{t8}")
                            ob = obs[t8]
                            ts_ = slice(t8 * 128, (t8 + 1) * 128)
                            os_ = slice(ot * 512, (ot + 1) * 512)
                            alt = last and ot % 2 == 1
                            pool = projps if alt else bigps
                            ps = pool.tile([128, 512], F32,
                                           tag="proj" if alt else "big")
                            prods = ((atthi, 0), (atthi, 1), (attlo, 0))
                            for pi, (at, wp) in enumerate(prods):
                                nc.tensor.matmul(
                                    ps[:],
                                    lhsT=at[:, :, ts_],
                                    rhs=wot_t[:, wp, :, os_],
                                    start=(pi == 0),
                                    stop=(pi == 2),
                                    perf_mode=DR,
                                )
                            with tc.high_priority():
                                if ot % 2 == 1:
                                    nc.scalar.copy(out=ob[:, ot], in_=ps[:])
                                else:
                                    nc.vector.tensor_copy(out=ob[:, ot], in_=ps[:])
                            rs = slice(s * SEQ + t8 * 128,
                                       s * SEQ + (t8 + 1) * 128)
                            if last and ot == 1:
                                # start streaming the final output early: the
                                # tail is gated by the serial DMA pipe
                                nc.scalar.dma_start(
                                    out=out[rs, 0:1024], in_=ob[:, 0:2])
                            elif last and ot == 3:
                                nc.scalar.dma_start(
                                    out=out[rs, 1024:2048], in_=ob[:, 2:4])
                            elif ot == 3:
                                nc.scalar.dma_start(out=out[rs, :], in_=ob[:])
                        fillers.append(emit)
                return fillers

            xtiles = {}

            def load_xtile(nt, eng=None):
                if nt in xtiles:
                    return xtiles[nt]
                eng = eng or nc.sync
                xtile = xtp.tile([128, 2, 16, 512], FP8, tag="xtile", name=f"xt{nt}")
                # hi plane first — the hi.hi pass only needs that half
                eng.dma_start(out=xtile[:, 0], in_=xt[:, 0, nt])
                eng.dma_start(out=xtile[:, 1], in_=xt[:, 1, nt])
                xtiles[nt] = xtile
                return xtile

            from collections import deque

            pend_fill = deque()
            for s in range(NSEQ):
                qk = qkp.tile([128, 4, SEQ], BF16, tag="qk", name=f"qk{s}")
                vt = vp.tile([128, 8, 256], BF16, tag="v", name=f"v{s}")
                atthi = attnp.tile([128, 2, SEQ], FP8, tag="atthi")
                attlo = attnp.tile([128, 2, SEQ], FP8, tag="attlo")
                gst = gsp.tile([128, 2, 8], BF16, tag="gst", name=f"gst{s}")
                if s == 0:
                    # JIT startup stream, all on the sync queue, ordered to
                    # match the pass-outer startup tile: whi-m0 + xhi unlock
                    # the hi.hi pass, then wvg-hi (v pass 1), xlo (pass 2),
                    # wvg-lo, and the wlo planes (pass 3) land while earlier
                    # passes execute.
                    xtile0 = xtp.tile([128, 2, 16, 512], FP8, tag="xtile", name="xt0")
                    nc.sync.dma_start(out=wqk_t[:, 0, 0], in_=wqk[:, 0, 0])
                    nc.sync.dma_start(out=xtile0[:, 0, 0:4, :], in_=xt[:, 0, 0, 0:4])
                    nc.sync.dma_start(out=wqk_t[:, 0, 1], in_=wqk[:, 0, 1])
                    nc.sync.dma_start(out=xtile0[:, 0, 4:10, :], in_=xt[:, 0, 0, 4:10])
                    nc.sync.dma_start(out=wqk_t[:, 0, 2], in_=wqk[:, 0, 2])
                    nc.sync.dma_start(out=xtile0[:, 0, 10:16, :], in_=xt[:, 0, 0, 10:16])
                    nc.sync.dma_start(out=wqk_t[:, 0, 3], in_=wqk[:, 0, 3])
                    nc.sync.dma_start(out=wvg_t[:, 0], in_=wvg[:, 0])
                    nc.sync.dma_start(out=xtile0[:, 1, 0:8, :], in_=xt[:, 1, 0, 0:8])
                    nc.sync.dma_start(out=xtile0[:, 1, 8:16, :], in_=xt[:, 1, 0, 8:16])
                    nc.sync.dma_start(out=wvg_t[:, 1], in_=wvg[:, 1])
                    nc.sync.dma_start(out=wqk_t[:, 1], in_=wqk[:, 1])
                    xtiles[0] = xtile0
                if s == 0:
                    qkv_ntile_startup(0, qk, vt, gst)
                    qkv_ntile_startup(1, qk, vt, gst)
                    late_consts()
                else:
                    qkv_ntile(2 * s, qk, vt, gst)
                    qkv_ntile(2 * s + 1, qk, vt, gst)
                gate_fin(s, gst)
                grow_t = rowp.tile([1, 2, 8, 128], BF16, tag="grow", name=f"grow{s}")
                nc.sync.dma_start(out=grow_t[:], in_=gate_scr[s])

                def fill(n=1):
                    k = 0
                    while pend_fill and k < n:
                        pend_fill.popleft()()
                        k += 1

                preps = []
                for h in range(2):
                    qf, _ = rope_norm(s, h, True, qk)
                    fill(1)
                    kf, kcol = rope_norm(s, h, False, qk)
                    fill(1)
                    preps.append((qf, kf, kcol))
                if s + 1 < NSEQ:
                    # prefetch next seq's x tiles AFTER the rope-swap DMAs:
                    # the swaps feed this seq's scores, the x tiles are not
                    # needed for another ~25us
                    load_xtile(2 * (s + 1))
                    load_xtile(2 * (s + 1) + 1)
                # the previous seq's Wo closures drain one-at-a-time at the
                # fill() points woven through this seq's stats + attention, so
                # the PE always has dependency-free work while the softmax
                # chains (exp on ACT, es on DVE) complete
                last = s == NSEQ - 1
                nxt = wo_fillers(s, atthi, attlo, last=last)
                if not last:
                    while pend_fill:
                        pend_fill.popleft()()
                    for h in range(2):
                        attention(s, h, qk, vt, atthi, attlo, *preps[h],
                                  grow_t, fill)
                    pend_fill = deque(nxt)
                else:
                    # last seq: drain the previous seq's closures during the
                    # first three phases, then this seq's own t8 0..3 (which
                    # only need the tt=0 att halves) fill the final phase
                    attention(s, 0, qk, vt, atthi, attlo, *preps[0],
                              grow_t, fill, tts_only=(0,))
                    attention(s, 1, qk, vt, atthi, attlo, *preps[1],
                              grow_t, fill, tts_only=(0,))
                    attention(s, 0, qk, vt, atthi, attlo, *preps[0],
                              grow_t, fill, tts_only=(1,))
                    while pend_fill:
                        pend_fill.popleft()()
                    pend_fill = deque(nxt[0:16])
                    attention(s, 1, qk, vt, atthi, attlo, *preps[1],
                              grow_t, fill, tts_only=(1,))
                    while pend_fill:
                        pend_fill.popleft()()
                    pend_fill = deque(nxt[16:32])
            while pend_fill:
                pend_fill.popleft()()

    if not nc.is_finalized():
        nc.finalize()
    return nc


_NC_CACHE = None


def _get_nc():
    global _NC_CACHE
    if _NC_CACHE is None:
        _NC_CACHE = build_nc()
    return _NC_CACHE


def _split_fp8(a, scale):
    """a (f32) -> (hi, lo) e4m3 planes of scale*a."""
    sa = (a * scale).astype(np.float32)
    hi = sa.astype(E4)
    lo = (sa - hi.astype(np.float32)).astype(E4)
    return hi, lo


def prep_inputs(x, Wqkv, Wo, gate_w, gate_b, norm_w, cos_cache, sin_cache,
                cu_seqlens, max_seqlen, position_ids):
    x = np.asarray(x, np.float32)
    Wqkv = np.asarray(Wqkv, np.float32)
    Wo = np.asarray(Wo, np.float32)
    gate_w = np.asarray(gate_w, np.float32)
    gate_b = np.asarray(gate_b, np.float32)
    norm_w = np.asarray(norm_w, np.float32)
    cos_cache = np.asarray(cos_cache, np.float32)
    sin_cache = np.asarray(sin_cache, np.float32)
    pid = np.asarray(position_ids).astype(np.int64)
    cu = np.asarray(cu_seqlens).astype(np.int64)
    assert int(max_seqlen) == SEQ and x.shape == (N_TOK, HID)
    assert np.array_equal(cu, np.arange(NSEQ + 1, dtype=np.int64) * SEQ)
    assert np.array_equal(pid, np.tile(np.arange(SEQ, dtype=np.int64), NSEQ))

    xtf = np.ascontiguousarray(x.T).reshape(16, 128, N_TOK).transpose(1, 0, 2)
    # [128, 16, N] -> [128, 8(tile), 16(kc), 512]
    xtf = np.ascontiguousarray(
        xtf.reshape(128, 16, 8, 512).transpose(0, 2, 1, 3)
    )
    xh, xl = _split_fp8(xtf, SX)
    xtf8 = np.ascontiguousarray(np.stack([xh, xl], axis=1))  # [128,2,8,16,512]

    C = cos_cache[pid[:SEQ]].T  # [64, 1024]
    S = sin_cache[pid[:SEQ]].T
    # sin plane stored [S; -S]: tmp = swap(qk) * sinplane gives
    # [x2*S; -x1*S] with no PE involvement
    csf = np.stack(
        [np.concatenate([C, C], 0), np.concatenate([S, -S], 0)], axis=1
    ).astype(BF)
    w2 = (norm_w * norm_w).reshape(128, 1).astype(np.float32)
    cskf = (csf.astype(np.float32) * w2[:, None, :]).astype(BF)

    trif = np.where(
        np.arange(128)[:, None] > np.arange(128)[None, :], np.float32(-1e30), 0.0
    ).astype(BF)
    idnf = np.eye(128, dtype=np.float32).astype(BF)

    in_maps = []
    for c in range(NCORES):
        hs = [2 * c, 2 * c + 1]
        rows = []
        for t in range(3):  # q, k, v row blocks of Wqkv
            for h in hs:
                rows.extend(range(t * HID + h * HD, t * HID + (h + 1) * HD))
        wsel = np.concatenate([Wqkv[rows], gate_w[hs]], axis=0)  # [770, 2048]
        wall = np.ascontiguousarray(wsel.T).reshape(16, 128, 770).transpose(1, 0, 2)
        wqk32 = np.ascontiguousarray(
            wall[:, :, 0:512].reshape(128, 16, 4, 128).transpose(0, 2, 1, 3)
        )  # [128, 4(m), 16(kc), 128]
        qh, ql = _split_fp8(wqk32, SW)
        wqkf = np.ascontiguousarray(np.stack([qh, ql], axis=1))
        wvg32 = np.ascontiguousarray(wall[:, :, 512:770])  # [128, 16, 258]
        vh, vl = _split_fp8(wvg32, SW)
        wvgf = np.ascontiguousarray(np.stack([vh, vl], axis=1))
        wo_sl = np.ascontiguousarray(Wo[:, c * 256 : (c + 1) * 256].T)
        wot32 = np.ascontiguousarray(wo_sl.reshape(2, 128, HID).transpose(1, 0, 2))
        oh, ol = _split_fp8(wot32, SW)
        wotf = np.ascontiguousarray(np.stack([oh, ol], axis=1))
        gbf = np.broadcast_to(-gate_b[hs][None, :], (128, 2)).astype(np.float32)
        gbf = np.ascontiguousarray(gbf)
        in_maps.append(
            {"xt": xtf8, "wqk": wqkf, "wvg": wvgf, "wot": wotf, "cs": csf,
             "tri": trif, "idn": idnf, "gbc": gbf, "csk": cskf}
        )
    return in_maps


def run(inputs, trace=False):
    in_maps = prep_inputs(**inputs)
    nc = _get_nc()
    res = run_bass_kernel_spmd(nc, in_maps, core_ids=list(range(NCORES)), trace=trace)
    total = np.zeros((N_TOK, HID), np.float32)
    for c in range(NCORES):
        total += res.results[c]["out"].astype(np.float32)
    total *= 1.0 / SOUT
    return total, res


def kernel(**inputs) -> np.ndarray:
    out, _ = run(inputs)
    return out


# revision 100
# speedup vs baseline: 1.0017x; 1.0017x over previous
"""Causal varlen self-attention (packed, equal-length) on 8 trn2 NeuronCores.

Sharding: tensor-parallel over heads — 16 heads / 8 cores = 2 heads per core.
Each core computes qkv + RoPE + RMSNorm + causal attention + sigmoid gating for
its 2 heads over all 4096 tokens, plus its partial output projection
(attn_chunk @ Wo_chunk.T).  The host sums the 8 partial outputs.

Matmul precision: the big projections (QKV, Wo) run as error-compensated fp8
DoubleRow matmuls: W = Whi + Wlo and x = xhi + xlo (each plane e4m3), and the
product is Whi·xhi + Whi·xlo + Wlo·xhi (the lo·lo term is ~0.03% and dropped).
That is 24 DR matmuls per 2048-deep contraction instead of 16 bf16 matmuls —
0.75x the PE time at slightly BETTER than bf16 accuracy.  Scales: x-planes
carry 2x, W-planes carry 64x, so qkv psum is 128x true.  q/k are RMS-
normalized downstream (scale cancels); the v/gate 128x folds into the gate
activation constants; the Wo result's 512x is divided out in the host-side
partial-sum reduction.

Per-core pipeline (feature-major q/k: head_dim on partitions):
  - qkv: q,k produced feature-major [d, t]; v (+ the 2 gate logits appended as
    2 extra columns of the v weight block) produced token-major [t, d].
  - RoPE via elementwise muls on a half-swapped copy; RMSNorm partition-
    reductions via ones-matmuls on the PE.
  - scores computed TRANSPOSED: scoresT[s, t] = k_fin-slices.T @ q_fin so the
    k-side softmax scale folds into the exp's per-partition scale, and the
    transposed probs are exactly what the PV matmul (lhsT = token-major V)
    wants.  Softmax denominator = ones-matmul over the exp tiles.
  - causal mask: diagonal-chunk matmuls are sliced to the unmasked t-range and
    one [128,128] triangle of -1e30 is added before exp.
  - gate and 1/denominator are per-token (free-dim) scales, applied via a
    partition-broadcast SBUF->SBUF DMA then one elementwise multiply.
"""

import sys

sys.path.insert(0, "/opt/trn_rl_repo")

import numpy as np
import ml_dtypes

import concourse.bass as bass
import concourse.tile as tile
from concourse import bacc, mybir
from concourse.bass_utils import run_bass_kernel_spmd

N_TOK, HID, NH, HD = 4096, 2048, 16, 128
SEQ, NSEQ = 1024, 4
NCORES = 8
EPS = 1e-6
F32, BF16, F32R = mybir.dt.float32, mybir.dt.bfloat16, mybir.dt.float32r
FP8 = mybir.dt.float8e4
BF = ml_dtypes.bfloat16
E4 = ml_dtypes.float8_e4m3
AF = mybir.ActivationFunctionType
DR = mybir.MatmulPerfMode.DoubleRow

SX = 2.0     # x-plane scale
SW = 64.0    # W-plane scale (qkv, wo)
SQKV = SX * SW          # qkv psum scale (128)
SATT = 8.0              # att sbuf scale
SOUT = SATT * SW        # wo psum scale (512), divided out on the host

_PATCHED = False
_ACT_PATCHED = False


def _patch_act_tables():
    """Force every ACT-table choice onto the one table that holds BOTH exp
    and ln (plus copy/identity/square): the load-insertion pass picks the
    first table containing each required func, and with distinct tables for
    Exp vs Ln the scheduler's interleaving causes a ~1.3us LoadActFuncSet
    per alternation.  Emptying every other entry (indices preserved, so the
    emitted act_func_set_id still matches act_info.json) makes the combined
    table the unique choice: exactly one load for the whole kernel."""
    global _ACT_PATCHED
    if _ACT_PATCHED:
        return
    _ACT_PATCHED = True
    import concourse.bacc as _bacc
    import concourse.bass_interp as _bi
    from concourse import mybir as _mb
    from concourse.hw_specs import get_activation_tables as _orig

    def patched(arch):
        tabs = _orig(arch)
        combo = {
            name
            for name, s in tabs.items()
            if _mb.ActivationFunctionType.Exp in s
            and _mb.ActivationFunctionType.Ln in s
        }
        assert combo, "no table with both Exp and Ln"
        return {
            name: (s if name in combo else set()) for name, s in tabs.items()
        }

    _bacc.get_activation_tables = patched
    _bi.get_activation_tables = patched


def _patch_tile_drain():
    """walrus in this env allows only ONE sync-wait on a TPB_CTRL instruction;
    spread the TileContext-exit drain's waits across nop instructions."""
    global _PATCHED
    if _PATCHED:
        return
    _PATCHED = True
    from concourse.tile import TileContext
    from concourse.vector_clock import ScopedClock

    def patched(self, tick_clock, wait_clock):
        nc = self.nc
        probe = nc.sync.nop(nofuse=True, hint="drain_waits_probe")
        wait_clock.add_sem_waits(probe.ins, ScopedClock({None: tick_clock.global_clock}))
        raw = list(probe.ins.sync_info.on_wait or [])
        best = {}
        for w in raw:  # keep one wait per semaphore (the largest threshold)
            k = (w.id, w.wait_mode)
            if k not in best or (w.wait_value or 0) > (best[k].wait_value or 0):
                best[k] = w
        waits = list(best.values())
        probe.ins.sync_info.on_wait = waits[:1]
        for w in waits[1:]:
            nop = nc.sync.nop(nofuse=True, hint="drain_waits")
            nop.ins.sync_info = mybir.SyncInfo(on_wait=[w], on_update=[])
        nc.sync.drain()
        nc.all_engine_barrier()
        assert self.sems is not None
        popped = nc._tile_sem_poison_stack.pop()
        assert popped is self._sem_poison
        nc.clear_and_free_semaphores(list(self.sems.allocated().values()))
        nc.all_engine_barrier()

    TileContext._drain_and_barrier = patched


def _r(ap):
    return ap.bitcast(F32R)


def build_nc():
    """One SPMD Bass program; all per-core data arrives via ExternalInputs."""
    _patch_act_tables()
    nc = bacc.Bacc("TRN2", target_bir_lowering=False, debug=False, num_devices=NCORES)

    # dim1 of xt/wqk/wvg/wot: 0 = hi fp8 plane, 1 = lo (residual) plane
    # xt dim2 = the 8 512-token tiles so a tile load is one contiguous
    # per-partition slab (DMA cost is row-count + bytes; 512B rows are 3x
    # slower than byte rate)
    xt = nc.dram_tensor("xt", [128, 2, 8, 16, 512], FP8, kind="ExternalInput")
    wqk = nc.dram_tensor("wqk", [128, 2, 4, 16, 128], FP8, kind="ExternalInput")
    wvg = nc.dram_tensor("wvg", [128, 2, 16, 258], FP8, kind="ExternalInput")
    wot = nc.dram_tensor("wot", [128, 2, 2, HID], FP8, kind="ExternalInput")
    cs = nc.dram_tensor("cs", [128, 2, SEQ], BF16, kind="ExternalInput")
    csk = nc.dram_tensor("csk", [128, 2, SEQ], BF16, kind="ExternalInput")
    tri = nc.dram_tensor("tri", [128, 128], BF16, kind="ExternalInput")
    idn = nc.dram_tensor("idn", [128, 128], BF16, kind="ExternalInput")
    gbc = nc.dram_tensor("gbc", [128, 2], F32, kind="ExternalInput")
    out = nc.dram_tensor("out", [N_TOK, HID], BF16, kind="ExternalOutput")
    gate_scr = nc.dram_tensor("gate_scr", [NSEQ, 2, 8, 128], BF16)

    with tile.TileContext(nc) as tc:
        with (
            tc.tile_pool(name="consts", bufs=1) as consts,
            tc.tile_pool(name="xtp", bufs=3) as xtp,
            tc.tile_pool(name="qkp", bufs=2) as qkp,
            tc.tile_pool(name="vp", bufs=2) as vp,
            tc.tile_pool(name="ropep", bufs=5) as ropep,
            tc.tile_pool(name="scrp", bufs=2) as scrp,
            tc.tile_pool(name="expp", bufs=3) as expp,
            tc.tile_pool(name="attnp", bufs=2) as attnp,
            tc.tile_pool(name="outp", bufs=3) as outp,
            tc.tile_pool(name="bcp", bufs=2) as bcp,
            tc.tile_pool(name="rowp", bufs=2) as rowp,
            tc.tile_pool(name="gsp", bufs=2) as gsp,
            tc.tile_pool(name="projps", bufs=2, space="PSUM") as projps,
            tc.tile_pool(name="bigps", bufs=3, space="PSUM") as bigps,
            tc.tile_pool(name="pvps", bufs=2, space="PSUM") as pvps,
            tc.tile_pool(name="vecps", bufs=1, space="PSUM") as vecps,
        ):
            # ---- resident constants
            wqk_t = consts.tile([128, 2, 4, 16, 128], FP8)
            wvg_t = consts.tile([128, 2, 16, 258], FP8)
            wot_t = consts.tile([128, 2, 2, HID], FP8)
            cs_t = consts.tile([128, 2, SEQ], BF16)
            csk_t = consts.tile([128, 2, SEQ], BF16)
            tri_t = consts.tile([128, 128], BF16)
            idn_t = consts.tile([128, 128], BF16)
            gbn_t = consts.tile([128, 2], F32)

            def late_consts():
                # on the same (sync) queue as the startup stream, emitted
                # after it: a second queue's HWDGE generations would
                # interleave 1:1 and dilute the startup bytes
                nc.sync.dma_start(out=cs_t[:], in_=cs[:])
                nc.sync.dma_start(out=csk_t[:], in_=csk[:])
                nc.sync.dma_start(out=tri_t[:], in_=tri[:])
                nc.sync.dma_start(out=idn_t[:], in_=idn[:])
                nc.sync.dma_start(out=gbn_t[:], in_=gbc[:])
                nc.sync.dma_start(out=wot_t[:], in_=wot[:])
            ones_t = consts.tile([128, 1], F32)
            nc.vector.memset(ones_t[:], 1.0)
            ones_bf = consts.tile([128, 1], BF16)
            nc.vector.memset(ones_bf[:], 1.0)
            # ones/HD in bf16 (2^-7, exact): the q-stats matmul yields mean_d
            ones_q = consts.tile([128, 1], BF16)
            nc.vector.memset(ones_q[:], 1.0 / HD)
            eps_t = consts.tile([128, 1], F32)
            nc.vector.memset(eps_t[:], EPS)
            epsh_t = consts.tile([128, 1], F32)
            # k-side stats run on SQKV-scaled k: eps scales by SQKV^2
            nc.vector.memset(epsh_t[:], float(HD * EPS * SQKV * SQKV))
            c16_t = consts.tile([128, 1], F32)
            nc.vector.memset(c16_t[:], float(SQKV / SATT))

            def qkv_mm(ps, m, wp, xp, c, start, stop):
                nc.tensor.matmul(
                    ps[:],
                    lhsT=wqk_t[:, wp, m, 2 * c : 2 * c + 2, :],
                    rhs=xp[:, 2 * c : 2 * c + 2, :],
                    start=start, stop=stop, perf_mode=DR,
                )

            def v_mm(ps, ti, wp, xp, c, start, stop):
                nc.tensor.matmul(
                    ps[:, 0:258],
                    lhsT=xp[:, 2 * c : 2 * c + 2, ti * 128 : (ti + 1) * 128],
                    rhs=wvg_t[:, wp, 2 * c : 2 * c + 2, :],
                    start=start, stop=stop, perf_mode=DR,
                )

            def qk_copy(qk, m, half, ps):
                dst = qk[:, m, half * 512 : (half + 1) * 512]
                with tc.high_priority():
                    nc.vector.tensor_copy(out=dst, in_=ps[:])

            def v_copy(vt, gst, half, ti, ps):
                with tc.high_priority():
                    nc.vector.tensor_copy(
                        out=vt[:, half * 4 + ti, :], in_=ps[:, 0:256])
                # stash the 2 gate-logit columns; exp'd per-seq in gate_fin
                with tc.high_priority():
                    nc.vector.tensor_copy(
                        out=gst[:, :, half * 4 + ti], in_=ps[:, 256:258]
                    )

            def qkv_ntile(nt, qk, vt, gst):
                """project 512 tokens: q,k feature-major; v+gate token-major.
                Each output = Whi.xhi + Whi.xlo + Wlo.xhi via fp8 DoubleRow."""
                half = nt % 2
                xtile = load_xtile(nt)
                xh, xl = xtile[:, 0], xtile[:, 1]
                for m in range(4):  # q_h0, q_h1, k_h0, k_h1
                    ps = projps.tile([128, 512], F32, tag="proj")
                    passes = ((0, xh), (0, xl), (1, xh))
                    for pi, (wp, xp) in enumerate(passes):
                        for c in range(8):
                            qkv_mm(ps, m, wp, xp, c,
                                   pi == 0 and c == 0, pi == 2 and c == 7)
                    qk_copy(qk, m, half, ps)
                for ti in range(4):  # v + gate logits, token-major, 128 tok each
                    ps = projps.tile([128, 512], F32, tag="proj")
                    passes = ((0, xh), (1, xh), (0, xl))
                    for pi, (wp, xp) in enumerate(passes):
                        for c in range(8):
                            v_mm(ps, ti, wp, xp, c,
                                 pi == 0 and c == 0, pi == 2 and c == 7)
                    v_copy(vt, gst, half, ti, ps)

            def qkv_ntile_startup(nt, qk, vt, gst):
                """pass-outer variant for the JIT startup tiles: all m/ti psum
                groups stay open across the three plane passes, so the PE only
                ever waits on the plane the current pass consumes (xhi first,
                then xlo, then the lo weights) instead of on the full tile."""
                half = nt % 2
                xtile = load_xtile(nt)
                xh, xl = xtile[:, 0], xtile[:, 1]
                mps = {}
                for m in range(4):
                    mps[m] = (projps if m < 2 else bigps).tile(
                        [128, 512], F32, tag="proj" if m < 2 else "big",
                        name=f"sqkv{nt}_{m}")
                    for c in range(8):
                        qkv_mm(mps[m], m, 0, xh, c, c == 0, False)
                vps = {}
                for ti in range(4):
                    vps[ti] = (pvps, bigps, vecps, pvps)[ti].tile(
                        [128, 512], F32,
                        tag=("pv", "big", "vec", "pv")[ti],
                        name=f"sv{nt}_{ti}")
                    for c in range(8):
                        v_mm(vps[ti], ti, 0, xh, c, c == 0, False)
                for m in range(4):
                    for c in range(8):
                        qkv_mm(mps[m], m, 0, xl, c, False, False)
                for ti in range(4):
                    for c in range(8):
                        v_mm(vps[ti], ti, 1, xh, c, False, False)
                for m in range(4):
                    for c in range(8):
                        qkv_mm(mps[m], m, 1, xh, c, False, c == 7)
                    qk_copy(qk, m, half, mps[m])
                for ti in range(4):
                    for c in range(8):
                        v_mm(vps[ti], ti, 0, xl, c, False, c == 7)
                    v_copy(vt, gst, half, ti, vps[ti])

            def gate_fin(s, gst):
                """gate rows hold (SQKV/SATT)*(1+exp(-(z+b))) for the whole
                seq, one DMA to DRAM; the reciprocal later turns this into
                SATT*sigmoid/(SQKV*den) so att lands at SATT*true given the
                SQKV-scaled pv."""
                for h in range(2):
                    nc.scalar.activation(
                        out=gst[:, h, :], in_=gst[:, h, :], func=AF.Exp,
                        bias=gbn_t[:, h : h + 1], scale=-1.0 / SQKV,
                    )
                # gst <- (gst + 1) * SQKV/SATT  == Copy((SQKV/SATT)*gst + 16)
                nc.scalar.activation(
                    out=gst[:, :, :], in_=gst[:, :, :], func=AF.Copy,
                    bias=float(SQKV / SATT), scale=float(SQKV / SATT),
                )
                nc.sync.dma_start(
                    out=gate_scr[s].rearrange("h ti p -> p h ti"), in_=gst[:]
                )

            def rope_norm(s, h, is_q, qk):
                """RoPE + RMSNorm scale for one head-tensor of one sequence.
                q: returns fin already scaled by sigma_q (broadcast multiply).
                k: returns (fin * norm_w^2, sigma_k per-partition column).
                All inputs are SQKV-scaled; sigma normalization cancels it."""
                m = h if is_q else 2 + h
                cst = cs_t if is_q else csk_t
                fin = ropep.tile([128, SEQ], BF16, tag="rope")
                sq = scrp.tile([128, SEQ], BF16, tag="sq")
                # rope is a per-pair rotation: it preserves sum_d q^2, so the
                # RMSNorm stats come from PRE-rope values — a chain parallel to
                # the rotation, not serial after it
                with tc.high_priority():
                    nc.vector.tensor_mul(out=sq[:], in0=qk[:, m, :], in1=qk[:, m, :])
                # half-swap via SBUF->SBUF DMA (the only cheap cross-partition
                # path besides the PE); sin plane is [S; -S] so the rotation is
                # then plain partition-aligned elementwise work
                qsw = scrp.tile([128, SEQ], BF16, tag="qsw")
                nc.sync.dma_start(out=qsw[0:64, :], in_=qk[64:128, m, :])
                nc.sync.dma_start(out=qsw[64:128, :], in_=qk[0:64, m, :])
                nc.vector.tensor_mul(out=qsw[:], in0=qsw[:], in1=cst[:, 1, :])
                for j in range(2):
                    js = slice(j * 512, (j + 1) * 512)
                    nc.gpsimd.tensor_mul(
                        out=fin[:, js], in0=qk[:, m, js], in1=cst[:, 0, js]
                    )
                    nc.vector.tensor_add(
                        out=fin[:, js], in0=fin[:, js], in1=qsw[:, js]
                    )
                if is_q:
                    # sigma_q[t] = rsqrt(mean_d(rope_q^2) + eps) computed as
                    # exp(-0.5*ln(x+eps)); the SQKV^2 in mean_d makes eps
                    # relatively smaller, otherwise identical.  free-dim
                    # scale, per 512-half so the first scores tile unblocks
                    # as early as possible
                    row = rowp.tile([1, SEQ], F32, tag="qrow")
                    for j in range(2):
                        js = slice(j * 512, (j + 1) * 512)
                        bc = bcp.tile([128, 512], F32, tag="bcq")
                        pss = vecps.tile([1, 512], F32, tag="vec")
                        nc.tensor.matmul(
                            pss[:],
                            lhsT=ones_q[:],
                            rhs=sq[:, js],
                            start=True,
                            stop=True,
                        )
                        # rsqrt as exp(-0.5*ln(x+eps)): keeps ACT on the one
                        # exp+ln table (zero LoadActFuncSet thrash)
                        nc.scalar.activation(
                            out=row[:, js], in_=pss[:], func=AF.Ln,
                            bias=eps_t[0:1, :], scale=1.0,
                        )
                        nc.scalar.activation(
                            out=row[:, js], in_=row[:, js], func=AF.Exp,
                            scale=-0.5,
                        )
                        nc.gpsimd.partition_broadcast(bc[:], row[:, js])
                        nc.vector.tensor_mul(
                            out=fin[:, js], in0=fin[:, js], in1=bc[:]
                        )
                    return fin, None
                else:
                    # sigma_k[s] = rsqrt(sum_d + HD*eps*SQKV^2), per-partition
                    # column applied inside the exp
                    col = rowp.tile([128, 8], F32, tag="kcol")
                    psc = projps.tile([128, 8], F32, tag="proj")
                    for sc in range(8):
                        nc.tensor.matmul(
                            psc[:, sc : sc + 1],
                            lhsT=sq[:, sc * 128 : (sc + 1) * 128],
                            rhs=ones_bf[:],
                            start=True,
                            stop=True,
                            skip_group_check=True,
                        )
                    nc.scalar.activation(
                        out=col[:], in_=psc[:], func=AF.Ln,
                        bias=epsh_t[:], scale=1.0,
                    )
                    nc.scalar.activation(
                        out=col[:], in_=col[:], func=AF.Exp, scale=-0.5,
                    )
                    return fin, col

            def attention(s, h, qk, vt, atthi, attlo, qf, kf, kcol,
                          grow_t, fill, tts_only=None):
                grows = [grow_t[0:1, h, 4 * tt : 4 * tt + 4, :] for tt in range(2)]
                for tt in range(2) if tts_only is None else tts_only:
                    nsc = 4 * (tt + 1)
                    expt = expp.tile([128, 8, 512], BF16, tag="expt")
                    pv = pvps.tile([128, 512], F32, tag="pv")
                    den = vecps.tile([1, 512], F32, tag="vec")
                    es = scrp.tile([128, 512], BF16, tag="esum")
                    for sc in range(nsc):
                        r = sc - 4 * tt  # >= 0 on diagonal chunks
                        c0 = 128 * r if r > 0 else 0
                        sps = bigps.tile([128, 512], F32, tag="big")
                        if r >= 0:  # diagonal chunk: lay down the -1e30
                            # triangle first so the (longer) score matmul is
                            # the one that releases the exp
                            nc.tensor.matmul(
                                sps[:, 0:128],
                                lhsT=idn_t[:],
                                rhs=tri_t[:],
                                start=True,
                                stop=False,
                                skip_group_check=True,
                            )
                        nc.tensor.matmul(
                            sps[:, 0 : 512 - c0],
                            lhsT=kf[:, sc * 128 : (sc + 1) * 128],
                            rhs=qf[:, tt * 512 + c0 : (tt + 1) * 512],
                            start=(r < 0),
                            stop=True,
                            skip_group_check=True,
                        )
                        with tc.high_priority():
                            nc.scalar.activation(
                                out=expt[:, sc, c0:512], in_=sps[:, 0 : 512 - c0],
                                func=AF.Exp, scale=kcol[:, sc : sc + 1],
                            )
                        # running bf16 partial sum of exp chunks on the DVE
                        # (2x mode) so the softmax denominator costs one
                        # [1,512] ones-matmul per tt instead of nsc of them
                        with tc.high_priority():
                            if sc == 0:
                                nc.vector.tensor_copy(out=es[:], in_=expt[:, 0, :])
                            else:
                                nc.vector.tensor_add(
                                    out=es[:, c0:512], in0=es[:, c0:512],
                                    in1=expt[:, sc, c0:512],
                                )
                        nc.tensor.matmul(
                            pv[:, c0:512],
                            lhsT=vt[:, sc, h * 128 : (h + 1) * 128],
                            rhs=expt[:, sc, c0:512],
                            start=(sc == 0),
                            stop=(sc == nsc - 1),
                            skip_group_check=True,
                        )
                        fill()
                    nc.tensor.matmul(
                        den[:], lhsT=ones_bf[:], rhs=es[:], start=True, stop=True,
                        skip_group_check=True,
                    )
                    fill()
                    drec = rowp.tile([1, 512], F32, tag="drec")
                    tts = slice(tt * 512, (tt + 1) * 512)
                    att = attnp.tile([128, 512], BF16, tag="attsc")
                    with tc.high_priority():
                        nc.vector.tensor_mul(out=drec[:], in0=den[:], in1=grows[tt][:])
                        nc.vector.reciprocal(out=drec[:], in_=drec[:])
                        bcg = bcp.tile([128, 512], F32, tag="bcg")
                        nc.gpsimd.partition_broadcast(bcg[:], drec[:])
                        nc.vector.tensor_mul(
                            out=att[:], in0=pv[:], in1=bcg[:]
                        )
                    # fp8 split for the DoubleRow Wo: hi = fp8(att),
                    # lo = fp8(att - hi); the sub runs on the (slack) Pool
                    # engine to keep it off the saturated DVE
                    with tc.high_priority():
                        nc.vector.tensor_copy(out=atthi[:, h, tts], in_=att[:])
                    nc.gpsimd.tensor_sub(
                        out=attlo[:, h, tts], in0=att[:],
                        in1=atthi[:, h, tts],
                    )

            def wo_fillers(s, atthi, attlo, last=False):
                """the previous seq's Wo projection as 32 (t8, ot) closures,
                emitted one at a time inside the next seq's attention to keep
                the PE instruction stream free of dependency stalls.  Each
                closure: 3 fp8 DoubleRow matmuls (hi.Whi + hi.Wlo + lo.Whi),
                contracting both heads at once via the [128, 2, t] att AP."""
                obs = {}
                fillers = []
                for t8 in range(8):
                    for ot in range(4):
                        def emit(t8=t8, ot=ot):
                            if ot == 0:
                                obs[t8] = outp.tile([128, 4, 512], BF16, tag="ob", name=f"ob{s}_{t8}")
                            ob = obs[t8]
                            ts_ = slice(t8 * 128, (t8 + 1) * 128)
                            os_ = slice(ot * 512, (ot + 1) * 512)
                            alt = last and ot % 2 == 1
                            pool = projps if alt else bigps
                            ps = pool.tile([128, 512], F32,
                                           tag="proj" if alt else "big")
                            prods = ((atthi, 0), (atthi, 1), (attlo, 0))
                            for pi, (at, wp) in enumerate(prods):
                                nc.tensor.matmul(
                                    ps[:],
                                    lhsT=at[:, :, ts_],
                                    rhs=wot_t[:, wp, :, os_],
                                    start=(pi == 0),
                                    stop=(pi == 2),
                                    perf_mode=DR,
                                )
                            with tc.high_priority():
                                if ot % 2 == 1:
                                    nc.scalar.copy(out=ob[:, ot], in_=ps[:])
                                else:
                                    nc.vector.tensor_copy(out=ob[:, ot], in_=ps[:])
                            rs = slice(s * SEQ + t8 * 128,
                                       s * SEQ + (t8 + 1) * 128)
                            if last and ot == 1:
                                # start streaming the final output early: the
                                # tail is gated by the serial DMA pipe
                                nc.scalar.dma_start(
                                    out=out[rs, 0:1024], in_=ob[:, 0:2])
                            elif last and ot == 3:
                                nc.scalar.dma_start(
                                    out=out[rs, 1024:2048], in_=ob[:, 2:4])
                            elif ot == 3:
                                nc.scalar.dma_start(out=out[rs, :], in_=ob[:])
                        fillers.append(emit)
                return fillers

            xtiles = {}

            def load_xtile(nt, eng=None):
                if nt in xtiles:
                    return xtiles[nt]
                eng = eng or nc.sync
                xtile = xtp.tile([128, 2, 16, 512], FP8, tag="xtile", name=f"xt{nt}")
                # hi plane first — the hi.hi pass only needs that half
                eng.dma_start(out=xtile[:, 0], in_=xt[:, 0, nt])
                eng.dma_start(out=xtile[:, 1], in_=xt[:, 1, nt])
                xtiles[nt] = xtile
                return xtile

            from collections import deque

            pend_fill = deque()
            for s in range(NSEQ):
                qk = qkp.tile([128, 4, SEQ], BF16, tag="qk", name=f"qk{s}")
                vt = vp.tile([128, 8, 256], BF16, tag="v", name=f"v{s}")
                atthi = attnp.tile([128, 2, SEQ], FP8, tag="atthi")
                attlo = attnp.tile([128, 2, SEQ], FP8, tag="attlo")
                gst = gsp.tile([128, 2, 8], BF16, tag="gst", name=f"gst{s}")
                if s == 0:
                    # JIT startup stream, all on the sync queue, ordered to
                    # match the pass-outer startup tile: whi-m0 + xhi unlock
                    # the hi.hi pass, then wvg-hi (v pass 1), xlo (pass 2),
                    # wvg-lo, and the wlo planes (pass 3) land while earlier
                    # passes execute.
                    xtile0 = xtp.tile([128, 2, 16, 512], FP8, tag="xtile", name="xt0")
                    nc.sync.dma_start(out=wqk_t[:, 0, 0], in_=wqk[:, 0, 0])
                    nc.sync.dma_start(out=xtile0[:, 0, 0:4, :], in_=xt[:, 0, 0, 0:4])
                    nc.sync.dma_start(out=wqk_t[:, 0, 1], in_=wqk[:, 0, 1])
                    nc.sync.dma_start(out=xtile0[:, 0, 4:10, :], in_=xt[:, 0, 0, 4:10])
                    nc.sync.dma_start(out=wqk_t[:, 0, 2], in_=wqk[:, 0, 2])
                    nc.sync.dma_start(out=xtile0[:, 0, 10:16, :], in_=xt[:, 0, 0, 10:16])
                    nc.sync.dma_start(out=wqk_t[:, 0, 3], in_=wqk[:, 0, 3])
                    nc.sync.dma_start(out=wvg_t[:, 0], in_=wvg[:, 0])
                    nc.sync.dma_start(out=xtile0[:, 1, 0:8, :], in_=xt[:, 1, 0, 0:8])
                    nc.sync.dma_start(out=xtile0[:, 1, 8:16, :], in_=xt[:, 1, 0, 8:16])
                    nc.sync.dma_start(out=wvg_t[:, 1], in_=wvg[:, 1])
                    nc.sync.dma_start(out=wqk_t[:, 1], in_=wqk[:, 1])
                    xtiles[0] = xtile0
                if s == 0:
                    qkv_ntile_startup(0, qk, vt, gst)
                    qkv_ntile_startup(1, qk, vt, gst)
                    late_consts()
                else:
                    qkv_ntile(2 * s, qk, vt, gst)
                    qkv_ntile(2 * s + 1, qk, vt, gst)
                gate_fin(s, gst)
                grow_t = rowp.tile([1, 2, 8, 128], BF16, tag="grow", name=f"grow{s}")
                nc.sync.dma_start(out=grow_t[:], in_=gate_scr[s])

                def fill(n=1):
                    k = 0
                    while pend_fill and k < n:
                        pend_fill.popleft()()
                        k += 1

                preps = []
                for h in range(2):
                    qf, _ = rope_norm(s, h, True, qk)
                    fill(1)
                    kf, kcol = rope_norm(s, h, False, qk)
                    fill(1)
                    preps.append((qf, kf, kcol))
                if s + 1 < NSEQ:
                    # prefetch next seq's x tiles AFTER the rope-swap DMAs:
                    # the swaps feed this seq's scores, the x tiles are not
                    # needed for another ~25us
                    load_xtile(2 * (s + 1))
                    load_xtile(2 * (s + 1) + 1)
                # the previous seq's Wo closures drain one-at-a-time at the
                # fill() points woven through this seq's stats + attention, so
                # the PE always has dependency-free work while the softmax
                # chains (exp on ACT, es on DVE) complete
                last = s == NSEQ - 1
                nxt = wo_fillers(s, atthi, attlo, last=last)
                if not last:
                    while pend_fill:
                        pend_fill.popleft()()
                    for h in range(2):
                        attention(s, h, qk, vt, atthi, attlo, *preps[h],
                                  grow_t, fill)
                    pend_fill = deque(nxt)
                else:
                    # last seq: drain the previous seq's closures during the
                    # first three phases, then this seq's own t8 0..3 (which
                    # only need the tt=0 att halves) fill the final phase
                    attention(s, 0, qk, vt, atthi, attlo, *preps[0],
                              grow_t, fill, tts_only=(0,))
                    attention(s, 1, qk, vt, atthi, attlo, *preps[1],
                              grow_t, fill, tts_only=(0,))
                    attention(s, 0, qk, vt, atthi, attlo, *preps[0],
                              grow_t, fill, tts_only=(1,))
                    while pend_fill:
                        pend_fill.popleft()()
                    pend_fill = deque(nxt[0:16])
                    attention(s, 1, qk, vt, atthi, attlo, *preps[1],
                              grow_t, fill, tts_only=(1,))
                    while pend_fill:
                        pend_fill.popleft()()
                    pend_fill = deque(nxt[16:32])
            while pend_fill:
                pend_fill.popleft()()

    if not nc.is_finalized():
        nc.finalize()
    return nc


_NC_CACHE = None


def _get_nc():
    global _NC_CACHE
    if _NC_CACHE is None:
        _NC_CACHE = build_nc()
    return _NC_CACHE


def _split_fp8(a, scale):
    """a (f32) -> (hi, lo) e4m3 planes of scale*a."""
    sa = (a * scale).astype(np.float32)
    hi = sa.astype(E4)
    lo = (sa - hi.astype(np.float32)).astype(E4)
    return hi, lo


def prep_inputs(x, Wqkv, Wo, gate_w, gate_b, norm_w, cos_cache, sin_cache,
                cu_seqlens, max_seqlen, position_ids):
    x = np.asarray(x, np.float32)
    Wqkv = np.asarray(Wqkv, np.float32)
    Wo = np.asarray(Wo, np.float32)
    gate_w = np.asarray(gate_w, np.float32)
    gate_b = np.asarray(gate_b, np.float32)
    norm_w = np.asarray(norm_w, np.float32)
    cos_cache = np.asarray(cos_cache, np.float32)
    sin_cache = np.asarray(sin_cache, np.float32)
    pid = np.asarray(position_ids).astype(np.int64)
    cu = np.asarray(cu_seqlens).astype(np.int64)
    assert int(max_seqlen) == SEQ and x.shape == (N_TOK, HID)
    assert np.array_equal(cu, np.arange(NSEQ + 1, dtype=np.int64) * SEQ)
    assert np.array_equal(pid, np.tile(np.arange(SEQ, dtype=np.int64), NSEQ))

    xtf = np.ascontiguousarray(x.T).reshape(16, 128, N_TOK).transpose(1, 0, 2)
    # [128, 16, N] -> [128, 8(tile), 16(kc), 512]
    xtf = np.ascontiguousarray(
        xtf.reshape(128, 16, 8, 512).transpose(0, 2, 1, 3)
    )
    xh, xl = _split_fp8(xtf, SX)
    xtf8 = np.ascontiguousarray(np.stack([xh, xl], axis=1))  # [128,2,8,16,512]

    C = cos_cache[pid[:SEQ]].T  # [64, 1024]
    S = sin_cache[pid[:SEQ]].T
    # sin plane stored [S; -S]: tmp = swap(qk) * sinplane gives
    # [x2*S; -x1*S] with no PE involvement
    csf = np.stack(
        [np.concatenate([C, C], 0), np.concatenate([S, -S], 0)], axis=1
    ).astype(BF)
    w2 = (norm_w * norm_w).reshape(128, 1).astype(np.float32)
    cskf = (csf.astype(np.float32) * w2[:, None, :]).astype(BF)

    trif = np.where(
        np.arange(128)[:, None] > np.arange(128)[None, :], np.float32(-1e30), 0.0
    ).astype(BF)
    idnf = np.eye(128, dtype=np.float32).astype(BF)

    in_maps = []
    for c in range(NCORES):
        hs = [2 * c, 2 * c + 1]
        rows = []
        for t in range(3):  # q, k, v row blocks of Wqkv
            for h in hs:
                rows.extend(range(t * HID + h * HD, t * HID + (h + 1) * HD))
        wsel = np.concatenate([Wqkv[rows], gate_w[hs]], axis=0)  # [770, 2048]
        wall = np.ascontiguousarray(wsel.T).reshape(16, 128, 770).transpose(1, 0, 2)
        wqk32 = np.ascontiguousarray(
            wall[:, :, 0:512].reshape(128, 16, 4, 128).transpose(0, 2, 1, 3)
        )  # [128, 4(m), 16(kc), 128]
        qh, ql = _split_fp8(wqk32, SW)
        wqkf = np.ascontiguousarray(np.stack([qh, ql], axis=1))
        wvg32 = np.ascontiguousarray(wall[:, :, 512:770])  # [128, 16, 258]
        vh, vl = _split_fp8(wvg32, SW)
        wvgf = np.ascontiguousarray(np.stack([vh, vl], axis=1))
        wo_sl = np.ascontiguousarray(Wo[:, c * 256 : (c + 1) * 256].T)
        wot32 = np.ascontiguousarray(wo_sl.reshape(2, 128, HID).transpose(1, 0, 2))
        oh, ol = _split_fp8(wot32, SW)
        wotf = np.ascontiguousarray(np.stack([oh, ol], axis=1))
        gbf = np.broadcast_to(-gate_b[hs][None, :], (128, 2)).astype(np.float32)
        gbf = np.ascontiguousarray(gbf)
        in_maps.append(
            {"xt": xtf8, "wqk": wqkf, "wvg": wvgf, "wot": wotf, "cs": csf,
             "tri": trif, "idn": idnf, "gbc": gbf, "csk": cskf}
        )
    return in_maps


def run(inputs, trace=False):
    in_maps = prep_inputs(**inputs)
    nc = _get_nc()
    res = run_bass_kernel_spmd(nc, in_maps, core_ids=list(range(NCORES)), trace=trace)
    total = np.zeros((N_TOK, HID), np.float32)
    for c in range(NCORES):
        total += res.results[c]["out"].astype(np.float32)
    total *= 1.0 / SOUT
    return total, res


def kernel(**inputs) -> np.ndarray:
    out, _ = run(inputs)
    return out
